# revision 4
# baseline (speedup 1.0000x reference)
"""CvT self-attention (depthwise-conv QKV projection + MHA) on 8 Trainium2 cores.

Sharding: data-parallel over batch B=64 -> 8 batches per core. No collectives.

Per-core pipeline (per batch, all matmuls fp16 w/ fp32 PSUM accumulation):
  1. DMA hidden [1025, 384] fp32, convert fp16, PE-transpose to channel-major
     x_pad [c, 34, 34] (zero-padded spatially).
  2. Depthwise 3x3 conv + folded BN as 9 diagonal-matmul taps accumulating in
     PSUM (q: stride 1, k/v: stride 2 via strided access patterns).
  3. QKV linear projections. q/k produce [c_out, tokens]; v is computed with
     conv output as the stationary operand producing token-major [t, c_out].
  4. Attention per head, scores TRANSPOSED ([t, l]) so no transpose is needed
     between softmax and PV: scoresT = kh^T qh, exp (no max subtraction --
     scores are O(1)), PV with ones-augmented V so the softmax denominator
     falls out of the same matmul, then PE-transpose [65, l] -> [l, 65] and
     normalize by the denominator column.
"""

import sys

sys.path.insert(0, "/opt/trn_rl_repo")

import numpy as np

import concourse.bass as bass
import concourse.mybir as mybir
import concourse.tile as tile
from concourse.masks import make_identity
from concourse.vector_clock import ScopedClock

B, C, H, W = 64, 384, 32, 32
NH, HD = 6, 64
L = 1 + H * W  # 1025 query tokens
TK = 1 + (H // 2) * (W // 2)  # 257 key/value tokens
NCORES = 8
BPC = B // NCORES  # batches per core
EPS = 1e-5
F16 = mybir.dt.float16
F32 = mybir.dt.float32
Act = mybir.ActivationFunctionType

TRACE = False
LAST_EXEC_NS = None

# l chunks for the 1025-token free dim (PSUM bank = 512 fp32)
LCH = [(0, 512), (512, 512), (1024, 1)]
# t chunks for the 257-token key dim over partitions
TCH = [(0, 128), (128, 128), (256, 1)]


def _split_multi_waits(nc):
    """walrus in this image only allows ONE sync wait per instruction. Move
    extra waits onto NoOps (same engine) inserted just before the offender."""
    from bass_rust import InstNoOp

    n_split = 0
    for blk in nc.m.functions[0].blocks:
        insts = blk.instructions
        out_list = []
        changed = False
        for inst in insts:
            si = inst.sync_info
            waits = list(si.on_wait) if si and si.on_wait else []
            if len(waits) > 1:
                changed = True
                for w in waits[:-1]:
                    n_split += 1
                    nop = InstNoOp(name=f"I-waitsplit-{n_split}", ins=[], outs=[])
                    nop.engine = inst.engine
                    nop.sync_info = mybir.SyncInfo(on_wait=[w], on_update=[])
                    out_list.append(nop)
                si.on_wait = waits[-1:]
            out_list.append(inst)
        if changed:
            blk.instructions = out_list


def _patch_drain():
    """Append wait-splitting to the end of TileContext's tail drain."""
    if getattr(tile.TileContext, "_drain_patched", False):
        return

    def _drain_and_barrier(self, tick_clock, wait_clock):
        nc = self.nc
        drain_inst = nc.sync.drain()
        wait_clock.add_sem_waits(
            drain_inst.ins, ScopedClock({None: tick_clock.global_clock})
        )
        nc.all_engine_barrier()
        assert self.sems is not None
        popped = nc._tile_sem_poison_stack.pop()
        assert popped is self._sem_poison
        nc.clear_and_free_semaphores(list(self.sems.allocated().values()))
        nc.all_engine_barrier()
        _split_multi_waits(nc)

    tile.TileContext._drain_and_barrier = _drain_and_barrier
    tile.TileContext._drain_patched = True


def _build_kernel():
    _patch_drain()
    nc = bass.Bass()
    hid = nc.dram_tensor("hid", [BPC, L, C], F32, kind="ExternalInput").ap()
    wdiag = nc.dram_tensor("wdiag", [128, 81, 128], F16, kind="ExternalInput").ap()
    wproj = nc.dram_tensor("wproj", [128, 18, 128], F16, kind="ExternalInput").ap()
    wpv = nc.dram_tensor("wpv", [128, 3, 384], F16, kind="ExternalInput").ap()
    biases = nc.dram_tensor("biases", [128, 15], F32, kind="ExternalInput").ap()
    out = nc.dram_tensor("out", [BPC, L, C], F32, kind="ExternalOutput").ap()

    with tile.TileContext(nc) as tc:
        with (
            tc.tile_pool(name="const", bufs=1) as const,
            tc.tile_pool(name="io", bufs=3) as io,
            tc.tile_pool(name="stage", bufs=2) as stage,
            tc.tile_pool(name="ctx", bufs=8) as ctxp,
            tc.tile_pool(name="outp", bufs=3) as outp,
            tc.tile_pool(name="small", bufs=4) as small,
            tc.tile_pool(name="pmm", bufs=4, space="PSUM") as pmm,
            tc.tile_pool(name="ptp", bufs=2, space="PSUM") as ptp,
            tc.tile_pool(name="pctx", bufs=2, space="PSUM") as pctx,
        ):
            # ---- constants ----
            wd_sb = const.tile([128, 81, 128], F16, tag="wd")
            nc.sync.dma_start(out=wd_sb[:], in_=wdiag)
            wp_sb = const.tile([128, 18, 128], F16, tag="wp")
            nc.sync.dma_start(out=wp_sb[:], in_=wproj)
            wpv_sb = const.tile([128, 3, 384], F16, tag="wpv")
            nc.sync.dma_start(out=wpv_sb[:], in_=wpv)
            bias_sb = const.tile([128, 15], F32, tag="bias")
            nc.sync.dma_start(out=bias_sb[:], in_=biases)
            ident = const.tile([128, 128], F16, tag="ident")
            make_identity(nc, ident[:])

            for b in range(BPC):
                # ---- stage A: load + transpose to channel-major ----
                x_pad = stage.tile([128, 3, 34, 34], F16, tag="xpad")
                # zero the 1-px border (interior is fully overwritten)
                nc.vector.memset(x_pad[:, :, 0, :], 0.0)
                nc.vector.memset(x_pad[:, :, 33, :], 0.0)
                nc.vector.memset(x_pad[:, :, 1:33, 0], 0.0)
                nc.vector.memset(x_pad[:, :, 1:33, 33], 0.0)

                for k in range(8):
                    x32 = io.tile([128, 384], F32, tag="x32")
                    nc.sync.dma_start(
                        out=x32[:], in_=hid[b, 1 + 128 * k : 1 + 128 * (k + 1), :]
                    )
                    x16 = io.tile([128, 384], F16, tag="x16")
                    nc.vector.tensor_copy(x16[:], x32[:])
                    for cc in range(3):
                        tp = ptp.tile([128, 128], F16, tag="tp")
                        nc.tensor.transpose(
                            tp[:], x16[:, cc * 128 : (cc + 1) * 128], ident[:]
                        )
                        # tokens 128k..128k+127 = image rows 4k..4k+3
                        nc.vector.tensor_copy(
                            x_pad[:, cc, 1 + 4 * k : 5 + 4 * k, 1:33], tp[:]
                        )

                cls32 = small.tile([128, 3], F32, tag="cls")
                for cc in range(3):
                    nc.sync.dma_start(
                        out=cls32[:, cc : cc + 1],
                        in_=hid[b, 0:1, cc * 128 : (cc + 1) * 128].rearrange(
                            "a b -> b a"
                        ),
                    )

                # ---- stage B: depthwise conv + BN (diagonal matmuls) ----
                q_src = stage.tile([128, 3, 1025], F16, tag="qsrc")
                k_src = stage.tile([128, 3, 257], F16, tag="ksrc")
                v_src = stage.tile([128, 3, 257], F16, tag="vsrc")
                for cc in range(3):
                    for s in (q_src, k_src, v_src):
                        nc.vector.tensor_copy(
                            s[:, cc, 0:1], cls32[:, cc : cc + 1]
                        )
                for cc in range(3):
                    # q: stride 1, two 512-token banks (16 image rows each)
                    for nb in range(2):
                        ps = pmm.tile([128, 512], F32, tag="mm")
                        for tap in range(9):
                            di, dj = tap // 3, tap % 3
                            rhs = x_pad[
                                :, cc, 16 * nb + di : 16 * nb + di + 16, dj : dj + 32
                            ]
                            nc.tensor.matmul(
                                ps[:],
                                wd_sb[:, tap * 3 + cc, :],
                                rhs,
                                start=(tap == 0),
                                stop=(tap == 8),
                            )
                        nc.scalar.activation(
                            q_src[:, cc, 1 + 512 * nb : 513 + 512 * nb],
                            ps[:],
                            Act.Identity,
                            bias=bias_sb[:, cc : cc + 1],
                        )
                    # k, v: stride 2 (16x16 outputs)
                    xv = x_pad[:, cc].rearrange(
                        "p (i ti) (j tj) -> p i ti j tj", ti=2, tj=2
                    )
                    for ci, src in ((1, k_src), (2, v_src)):
                        ps = pmm.tile([128, 512], F32, tag="mm")
                        for tap in range(9):
                            di, dj = tap // 3, tap % 3
                            rhs = xv[
                                :,
                                di // 2 : di // 2 + 16,
                                di % 2,
                                dj // 2 : dj // 2 + 16,
                                dj % 2,
                            ]
                            nc.tensor.matmul(
                                ps[:, :256],
                                wd_sb[:, ci * 27 + tap * 3 + cc, :],
                                rhs,
                                start=(tap == 0),
                                stop=(tap == 8),
                            )
                        nc.scalar.activation(
                            src[:, cc, 1:257],
                            ps[:, :256],
                            Act.Identity,
                            bias=bias_sb[:, ci * 3 + cc : ci * 3 + cc + 1],
                        )

                # ---- stage C: projections ----
                qh = stage.tile([128, 3, 1025], F16, tag="qh")
                kh = stage.tile([128, 3, 257], F16, tag="kh")
                for mc in range(3):
                    for n0, nl in LCH:
                        ps = pmm.tile([128, 512], F32, tag="mm")
                        for kc in range(3):
                            nc.tensor.matmul(
                                ps[:, :nl],
                                wp_sb[:, kc * 3 + mc, :],
                                q_src[:, kc, n0 : n0 + nl],
                                start=(kc == 0),
                                stop=(kc == 2),
                            )
                        nc.vector.tensor_scalar_add(
                            qh[:, mc, n0 : n0 + nl],
                            ps[:, :nl],
                            bias_sb[:, 9 + mc : 10 + mc],
                        )
                    ps = pmm.tile([128, 512], F32, tag="mm")
                    for kc in range(3):
                        nc.tensor.matmul(
                            ps[:, :257],
                            wp_sb[:, 9 + kc * 3 + mc, :],
                            k_src[:, kc, :],
                            start=(kc == 0),
                            stop=(kc == 2),
                        )
                    nc.vector.tensor_scalar_add(
                        kh[:, mc, :], ps[:, :257], bias_sb[:, 12 + mc : 13 + mc]
                    )
                v_store = stage.tile([128, 3, 6, 65], F16, tag="vst")
                nc.vector.memset(v_store[:, :, :, 64:65], 1.0)
                for tcc, (t0, tl) in enumerate(TCH):
                    ps = pmm.tile([128, 512], F32, tag="mm")
                    for kc in range(3):
                        nc.tensor.matmul(
                            ps[:tl, :384],
                            v_src[:, kc, t0 : t0 + tl],
                            wpv_sb[:, kc, :],
                            start=(kc == 0),
                            stop=(kc == 2),
                        )
                    nc.vector.tensor_copy(
                        v_store[:tl, tcc, :, 0:64],
                        ps[:tl, :384].rearrange("p (h d) -> p h d", h=6),
                    )

                # ---- stage D/E: attention per head ----
                ctx_tiles = []
                for h in range(6):
                    base, ch = (h % 2) * 64, h // 2
                    expT = stage.tile([128, 3, 1025], F16, tag="expT")
                    for tcc, (t0, tl) in enumerate(TCH):
                        for n0, nl in LCH:
                            ps = pmm.tile([128, 512], F32, tag="mm")
                            nc.tensor.matmul(
                                ps[:tl, :nl],
                                kh[base : base + 64, ch, t0 : t0 + tl],
                                qh[base : base + 64, ch, n0 : n0 + nl],
                                start=True,
                                stop=True,
                            )
                            nc.scalar.activation(
                                expT[:tl, tcc, n0 : n0 + nl], ps[:tl, :nl], Act.Exp
                            )
                    ctxsb = ctxp.tile([128, 1025], F16, tag="ctx")
                    ctx_tiles.append(ctxsb)
                    for n0, nl in LCH:
                        cps = pctx.tile([65, 512], F32, tag="pc")
                        for tcc, (t0, tl) in enumerate(TCH):
                            nc.tensor.matmul(
                                cps[:, :nl],
                                v_store[:tl, tcc, h, :],
                                expT[:tl, tcc, n0 : n0 + nl],
                                start=(tcc == 0),
                                stop=(tcc == 2),
                            )
                        nc.scalar.activation(
                            ctxsb[:65, n0 : n0 + nl], cps[:, :nl], Act.Copy
                        )

                # ---- stage F: transpose back, normalize, store ----
                for lc in range(9):
                    l0 = lc * 128
                    ll = min(128, L - l0)
                    osb = outp.tile([128, 384], F32, tag="osb")
                    for h in range(6):
                        tp = ptp.tile([128, 65], F16, tag="tp")
                        nc.tensor.transpose(
                            tp[:ll, :],
                            ctx_tiles[h][:65, l0 : l0 + ll],
                            ident[:65, :65],
                        )
                        rec = small.tile([128, 1], F32, tag="rec")
                        nc.vector.reciprocal(rec[:ll], tp[:ll, 64:65])
                        nc.vector.tensor_scalar_mul(
                            osb[:ll, h * 64 : (h + 1) * 64], tp[:ll, 0:64], rec[:ll]
                        )
                    nc.sync.dma_start(
                        out=out[b, l0 : l0 + ll, :], in_=osb[:ll, :]
                    )
    return nc


def _install_trace_support():
    """Provide the NTFF profile hook (this image's antenv lacks axon_hooks)
    and neuter the artifact upload (no fish access here)."""
    import contextlib
    import ctypes
    import types

    import concourse.bass_utils as bu

    bu.upload_artifacts = lambda tmpdir: f"local:{tmpdir}"
    try:
        from antenv.axon_hooks import get_axon_ntff_profile_hook  # noqa: F401

        return
    except ImportError:
        pass
    so_path = "/opt/axon/libaxon_pjrt.so"
    lib = ctypes.CDLL(so_path)
    if not hasattr(lib, "axon_start_nrt_profile"):
        return
    lib.axon_start_nrt_profile.argtypes = [
        ctypes.POINTER(ctypes.c_int64),
        ctypes.c_size_t,
    ]
    lib.axon_start_nrt_profile.restype = ctypes.c_int64
    lib.axon_stop_nrt_profile.argtypes = [ctypes.c_char_p]
    lib.axon_stop_nrt_profile.restype = ctypes.c_int64

    @contextlib.contextmanager
    def _hook(output_dir, device_ids):
        import jax

        jax.devices()
        if device_ids:
            ids = (ctypes.c_int64 * len(device_ids))(*device_ids)
            rc = lib.axon_start_nrt_profile(ids, len(device_ids))
        else:
            rc = lib.axon_start_nrt_profile(None, 0)
        if rc != 0:
            raise RuntimeError(f"axon_start_nrt_profile rc={rc}")
        try:
            yield
        finally:
            n = lib.axon_stop_nrt_profile(str(output_dir).encode())
            print(f"profile: {n} file(s) written to {output_dir}")

    import antenv

    mod = types.ModuleType("antenv.axon_hooks")
    holder = {"h": _hook}
    mod.get_axon_ntff_profile_hook = lambda: holder["h"]
    mod.set_axon_ntff_profile_hook = lambda h: holder.__setitem__("h", h)
    antenv.axon_hooks = mod
    sys.modules["antenv.axon_hooks"] = mod


_CACHED = None


def _prep_weights(inputs):
    """Fold BN into conv weights; pre-transpose/chunk projection weights."""
    f16 = np.float16
    wdiag = np.zeros((128, 81, 128), dtype=f16)
    biases = np.zeros((128, 15), dtype=np.float32)
    wproj = np.zeros((128, 18, 128), dtype=f16)
    wpv = np.zeros((128, 3, 384), dtype=f16)
    for ci, p in enumerate(["q", "k", "v"]):
        gamma = np.asarray(inputs[f"bn_{p}_gamma"], np.float64)
        var = np.asarray(inputs[f"bn_{p}_var"], np.float64)
        beta = np.asarray(inputs[f"bn_{p}_beta"], np.float64)
        mean = np.asarray(inputs[f"bn_{p}_mean"], np.float64)
        inv = gamma / np.sqrt(var + EPS)
        wfold = np.asarray(inputs[f"conv_{p}_w"], np.float64)[:, 0] * inv[:, None, None]
        bias_c = beta - mean * inv
        for tap in range(9):
            di, dj = tap // 3, tap % 3
            for cc in range(3):
                d = wfold[cc * 128 : (cc + 1) * 128, di, dj]
                np.fill_diagonal(wdiag[:, ci * 27 + tap * 3 + cc, :], d.astype(f16))
        for cc in range(3):
            biases[:, ci * 3 + cc] = bias_c[cc * 128 : (cc + 1) * 128]
        w = np.asarray(inputs[f"w_{p}"], np.float64)  # [o, c]
        assert np.abs(np.asarray(inputs[f"b_{p}"])).max() == 0.0 or p != "v", (
            "nonzero v bias unsupported"
        )
        if p == "q":
            wt = (w.T * (C**-0.5)).astype(f16)  # fold attention scale
        else:
            wt = w.T.astype(f16)
        if p in ("q", "k"):
            pi = 0 if p == "q" else 1
            for kc in range(3):
                for mc in range(3):
                    wproj[:, pi * 9 + kc * 3 + mc, :] = wt[
                        kc * 128 : (kc + 1) * 128, mc * 128 : (mc + 1) * 128
                    ]
            # projection bias (spec: zeros, but supported per out-channel)
            bvec = np.asarray(inputs[f"b_{p}"], np.float64) * (
                (C**-0.5) if p == "q" else 1.0
            )
            for mc in range(3):
                biases[:, 9 + pi * 3 + mc] = bvec[mc * 128 : (mc + 1) * 128]
        else:
            for kc in range(3):
                wpv[:, kc, :] = wt[kc * 128 : (kc + 1) * 128, :]
    return wdiag, wproj, wpv, biases


def kernel(**inputs) -> np.ndarray:
    global _CACHED, LAST_EXEC_NS
    from concourse.bass_utils import run_bass_kernel_spmd

    if TRACE:
        _install_trace_support()
    hidden = np.ascontiguousarray(np.asarray(inputs["hidden_state"], np.float32))
    assert hidden.shape == (B, L, C)
    wdiag, wproj, wpv, biases = _prep_weights(inputs)

    if _CACHED is None:
        _CACHED = _build_kernel()
    nc = _CACHED

    in_maps = []
    for core in range(NCORES):
        in_maps.append(
            {
                "hid": hidden[core * BPC : (core + 1) * BPC],
                "wdiag": wdiag,
                "wproj": wproj,
                "wpv": wpv,
                "biases": biases,
            }
        )
    res = run_bass_kernel_spmd(
        nc, in_maps, core_ids=list(range(NCORES)), trace=TRACE
    )
    LAST_EXEC_NS = res.exec_time_ns
    out = np.concatenate([res.results[i]["out"] for i in range(NCORES)], axis=0)
    return out.astype(np.float32)
